# revision 7
# baseline (speedup 1.0000x reference)
"""Distributed Trainium2 Bass kernel for AtnConv (contextual-attention conv).

Everything runs on device; the tunnel carries only compact inputs and the
final output. 8 cores = batch(2) x quarter(4). Within a sample group of 4:
  - x1^T and x2 (bf16, padded) are uploaded as quarter-shards and AllGathered
    device-side (HBM-HBM over NeuronLink).
  - Each core owns 1024 of the 4096 positions: scores = cols_q^T @ cols,
    scaled in f32 by SCALE*mm/norm, local softmax over all L, exact mask
    multiply + 1e-8 clamp on device.
  - U[c',pos] = R'^T Y via PE (R' streamed straight out of gathered x1^T, so
    col2im consumes U blocks per (di,dj) with no reshuffle), scatter-added
    into a 36-row window; windows AllGathered, every core assembles full y.
  - Final 4 dilated convs: 33-shift union with per-core weight data (zeros
    for foreign rates) keeps the program SPMD-uniform; each core emits only
    its rate's 16 channels [16,128,128] bf16.
Host does only padding/transpose/casts and output concat.
"""

import numpy as np
import ml_dtypes


def _enable_jax_compilation_cache():
    # run_bass_kernel_spmd builds a fresh jit closure per call, so JAX's
    # in-process executable cache never hits and every dispatch re-runs the
    # BIR->NEFF compile (~0.8s). The persistent cache keys on the (stable)
    # serialized HLO and skips that.
    try:
        import jax
        jax.config.update("jax_compilation_cache_dir", "/tmp/jax_comp_cache")
        jax.config.update("jax_persistent_cache_min_compile_time_secs", 0)
        jax.config.update("jax_persistent_cache_min_entry_size_bytes", -1)
    except Exception:
        pass


_enable_jax_compilation_cache()

B, C, H1, H2 = 2, 128, 128, 64
L = H2 * H2            # 4096 patches / positions
POSL = 1024            # positions per core
SCALE = 10.0
EPS_NORM = 1e-4
EPS_CLAMP = 1e-8
RATES = (1, 2, 4, 8)
SHIFTS = sorted({(r * (u - 1), r * (v - 1))
                 for r in RATES for u in range(3) for v in range(3)})
NSH = len(SHIFTS)      # 33
BF16 = ml_dtypes.bfloat16
GROUPS = [[0, 1, 2, 3], [4, 5, 6, 7]]

X1CH = 130 * 130 * 128 // 4   # 540800 bf16 elems per x1 shard
X2QCH = 128 * 18 * 66         # one overlapping 18-row x2 chunk (halo 1)
# bf16 blob layout (element offsets)
OFF_X1 = 0
OFF_X2Q = OFF_X1 + X1CH
OFF_MM = OFF_X2Q + X2QCH
OFF_FW = OFF_MM + L
OFF_SCHI = OFF_FW + NSH * 128 * 16
OFF_SCLO = OFF_SCHI + L
OFF_FBHI = OFF_SCLO + L
OFF_FBLO = OFF_FBHI + 16
BFBLOB = OFF_FBLO + 16

_NC = None


def _build_nc():
    import concourse.bass as bass
    import concourse.bacc as bacc
    import concourse.mybir as mybir
    from concourse import tile

    bf = mybir.dt.bfloat16
    f32 = mybir.dt.float32
    Exp = mybir.ActivationFunctionType.Exp
    Relu = mybir.ActivationFunctionType.Relu
    X = mybir.AxisListType.X
    AG = "AllGather"
    BYP = mybir.AluOpType.bypass

    nc = bacc.Bacc(None, target_bir_lowering=False)
    p_bf = nc.declare_dram_parameter("p_bf", [BFBLOB], bf, isOutput=False)
    outp = nc.declare_dram_parameter("outp", [16, 128, 128], bf, isOutput=True)

    with tile.TileContext(nc) as tc:
        with (
            tc.tile_pool(name="dram", bufs=1, space="DRAM") as dram,
            tc.tile_pool(name="st", bufs=1) as st,
            tc.tile_pool(name="fin", bufs=2) as fin,
        ):
            # ---- kick off input gathers first (overlap with local prep) ----
            b_x2q = dram.tile([128, 18, 66], bf)
            g_x2q = dram.tile([4, 128, 18, 66], bf)
            b_x1 = dram.tile([X1CH], bf)
            g_x1 = dram.tile([130, 130, 128], bf)
            b_w = dram.tile([128, 36, 130], f32)
            g_w = dram.tile([4, 128, 36, 130], f32)
            d_fw = dram.tile([NSH, 128, 16], bf)
            nc.gpsimd.dma_start(b_x2q[:], p_bf[OFF_X2Q:OFF_X2Q + X2QCH])
            nc.gpsimd.collective_compute(AG, BYP, replica_groups=GROUPS,
                                         ins=[b_x2q[:]], outs=[g_x2q[:]])
            nc.gpsimd.dma_start(b_x1[:], p_bf[OFF_X1:OFF_X1 + X1CH])
            nc.gpsimd.dma_start(d_fw[:], p_bf[OFF_FW:OFF_FW + NSH * 128 * 16])
            nc.gpsimd.collective_compute(AG, BYP, replica_groups=GROUPS,
                                         ins=[b_x1[:]], outs=[g_x1[:]])

            # ---- persistent small state ----
            nbmaxs = st.tile([128, 8, 8], f32)
            rss = st.tile([128, 8, 8], f32)
            mmb = st.tile([128, L], bf)
            nc.sync.dma_start(mmb[0:1, :], p_bf[OFF_MM:OFF_MM + L])
            p = 1
            while p < 128:
                nc.sync.dma_start(mmb[p:2 * p, :], mmb[0:p, :])
                p *= 2

            with tc.tile_pool(name="estp", bufs=1) as estp:
                estore = estp.tile([128, 8, L], bf)   # Y^T, 64 KiB/part

                # ---- scores + block-local softmax ----
                with (
                    tc.tile_pool(name="ph1", bufs=1) as ph1,
                    tc.tile_pool(name="wka", bufs=2) as wka,
                    tc.tile_pool(name="psa", bufs=2, space=bass.MemorySpace.PSUM) as psa,
                ):
                    xt = ph1.tile([128, 9, 16, 64], bf)
                    scb = ph1.tile([128, L], f32)
                    for u in range(3):
                        for v in range(3):
                            nc.sync.dma_start(xt[:, 3 * u + v],
                                              b_x2q[:, u:u + 16, v:v + 64])
                    sc_hi = ph1.tile([1, L], bf)
                    sc_lo = ph1.tile([1, L], bf)
                    nc.sync.dma_start(sc_hi[:], p_bf[OFF_SCHI:OFF_SCHI + L])
                    nc.sync.dma_start(sc_lo[:], p_bf[OFF_SCLO:OFF_SCLO + L])
                    nc.vector.tensor_add(scb[0:1, :], sc_hi[:], sc_lo[:])
                    p = 1
                    while p < 128:
                        nc.sync.dma_start(scb[p:2 * p, :], scb[0:p, :])
                        p *= 2

                    for n in range(8):            # L blocks of 512 (8 i-rows)
                        a_n = wka.tile([128, 9, 8, 64], bf, tag="a_n")
                        ch = n // 2
                        r0 = 8 * n - 16 * ch
                        for u in range(3):
                            for v in range(3):
                                nc.sync.dma_start(
                                    a_n[:, 3 * u + v],
                                    g_x2q[ch][:, r0 + u:r0 + u + 8, v:v + 64])
                        for m in range(8):        # pos tiles of 128
                            z = psa.tile([128, 512], f32, tag="z")
                            for k in range(9):
                                nc.tensor.matmul(z[:], xt[:, k, 2 * m:2 * m + 2, :],
                                                 a_n[:, k], start=(k == 0),
                                                 stop=(k == 8))
                            zs = wka.tile([128, 512], f32, tag="zs")
                            nc.vector.tensor_mul(zs[:], z[:],
                                                 scb[:, n * 512:(n + 1) * 512])
                            nc.vector.reduce_max(nbmaxs[:, m, n:n + 1], zs[:],
                                                 axis=X, negate=True)
                            ef = wka.tile([128, 512], f32, tag="ef")
                            nc.scalar.activation(ef[:], zs[:], Exp,
                                                 bias=nbmaxs[:, m, n:n + 1],
                                                 scale=1.0)
                            nc.vector.reduce_sum(rss[:, m, n:n + 1], ef[:], axis=X)
                            nc.vector.tensor_copy(
                                estore[:, m, n * 512:(n + 1) * 512], ef[:])

                # ---- softmax finalize + exact mask & clamp ----
                for m in range(8):
                    ngm = fin.tile([128, 1], f32, tag="ngm")
                    nc.vector.tensor_reduce(ngm[:], nbmaxs[:, m, :], axis=X,
                                            op=mybir.AluOpType.min)
                    al = fin.tile([128, 8], f32, tag="al")
                    nc.scalar.activation(al[:], nbmaxs[:, m, :], Exp, bias=ngm[:],
                                         scale=-1.0)
                    pr = fin.tile([128, 8], f32, tag="pr")
                    nc.vector.tensor_mul(pr[:], al[:], rss[:, m, :])
                    sm = fin.tile([128, 1], f32, tag="sm")
                    nc.vector.reduce_sum(sm[:], pr[:], axis=X)
                    rc = fin.tile([128, 1], f32, tag="rc")
                    nc.vector.reciprocal(rc[:], sm[:])
                    be = fin.tile([128, 8], f32, tag="be")
                    nc.vector.tensor_scalar_mul(be[:], al[:], rc[:])
                    for n in range(8):
                        nc.vector.tensor_scalar_mul(
                            estore[:, m, n * 512:(n + 1) * 512],
                            estore[:, m, n * 512:(n + 1) * 512], be[:, n:n + 1])
                    nc.vector.tensor_mul(estore[:, m, :], estore[:, m, :], mmb[:])
                    nc.vector.tensor_scalar_max(estore[:, m, :], estore[:, m, :],
                                                EPS_CLAMP)

                # ---- U = R'^T Y per pos-half, col2im into window ----
                with tc.tile_pool(name="wpool", bufs=1) as wpool:
                    window = wpool.tile([128, 36, 130], f32)
                    nc.vector.memset(window[:], 0.0)
                    for half in range(2):
                        with (
                            tc.tile_pool(name="ybh", bufs=1) as ybh,
                            tc.tile_pool(name="wkc", bufs=2) as wkc,
                            tc.tile_pool(name="psb", bufs=1,
                                         space=bass.MemorySpace.PSUM) as psb,
                        ):
                            ybufT = ybh.tile([128, 32, 512], bf)
                            for mloc in range(4):
                                m = 4 * half + mloc
                                for kk in range(32):
                                    nc.sync.dma_start_transpose(
                                        ybufT[:, kk, mloc * 128:(mloc + 1) * 128],
                                        estore[:, m, kk * 128:(kk + 1) * 128])
                            for gg in range(4):
                                ups = [psb.tile([128, 8, 64], f32, tag=f"u{j}",
                                                name=f"ups{j}")
                                       for j in range(4)]
                                for k in range(32):
                                    rt = wkc.tile([128, 4, 128], bf, tag="rt")
                                    for j in range(4):
                                        g = 4 * gg + j
                                        di, dj = divmod(g, 4)
                                        nc.sync.dma_start(
                                            rt[:, j],
                                            g_x1[4 * k + di:4 * k + di + 3:2,
                                                 dj:dj + 127:2, :])
                                    for j in range(4):
                                        nc.tensor.matmul(ups[j][:], rt[:, j],
                                                         ybufT[:, k, :],
                                                         start=(k == 0),
                                                         stop=(k == 31))
                                for j in range(4):
                                    g = 4 * gg + j
                                    di, dj = divmod(g, 4)
                                    r0 = di + 1 + 16 * half
                                    sl = window[:, r0:r0 + 15:2, dj:dj + 127:2]
                                    nc.vector.tensor_add(sl, sl, ups[j][:])
                    nc.gpsimd.dma_start(b_w[:], window[:])

            # ---- gather windows, assemble y, final dilated convs ----
            nc.gpsimd.collective_compute(AG, BYP, replica_groups=GROUPS,
                                         ins=[b_w[:]], outs=[g_w[:]])
            with (
                tc.tile_pool(name="convp", bufs=1) as convp,
                tc.tile_pool(name="wkd", bufs=2) as wkd,
                tc.tile_pool(name="psc", bufs=2,
                             space=bass.MemorySpace.PSUM) as psc,
            ):
                y_bf = convp.tile([128, 144, 144], bf)
                fw_sb = convp.tile([128, NSH, 16], bf)
                fb_sb = convp.tile([16, 1], f32)
                for si in range(NSH):
                    nc.sync.dma_start(fw_sb[:, si, :], d_fw[si])
                fb_hi = convp.tile([16, 1], bf)
                fb_lo = convp.tile([16, 1], bf)
                nc.sync.dma_start(fb_hi[:], p_bf[OFF_FBHI:OFF_FBHI + 16])
                nc.sync.dma_start(fb_lo[:], p_bf[OFF_FBLO:OFF_FBLO + 16])
                nc.vector.tensor_add(fb_sb[:], fb_hi[:], fb_lo[:])
                with tc.tile_pool(name="ypool", bufs=1) as ypool:
                    y_buf = ypool.tile([128, 144, 144], f32)
                    nc.vector.memset(y_buf[:], 0.0)
                    for k in range(4):
                        wstg = wkd.tile([128, 36, 130], f32, tag="wstg")
                        nc.gpsimd.dma_start(wstg[:], g_w[k])
                        t0 = 2 if k == 0 else 1
                        t1 = 34 if k == 3 else 35
                        dst = y_buf[:, 32 * k + 6 + t0:32 * k + 6 + t1, 8:136]
                        nc.vector.tensor_add(dst, dst, wstg[:, t0:t1, 1:129])
                    nc.vector.tensor_copy(y_bf[:], y_buf[:])
                for blk in range(32):             # out row blocks of 4
                    ops = psc.tile([16, 4, 128], f32, tag="ops")
                    for si, (dh, dv) in enumerate(SHIFTS):
                        r0 = 8 + dh + 4 * blk
                        nc.tensor.matmul(ops[:], fw_sb[:, si, :],
                                         y_bf[:, r0:r0 + 4, 8 + dv:8 + dv + 128],
                                         start=(si == 0), stop=(si == NSH - 1))
                    ob = wkd.tile([16, 4, 128], bf, tag="ob")
                    nc.scalar.activation(ob[:], ops[:], Relu, bias=fb_sb[:],
                                         scale=1.0)
                    nc.sync.dma_start(outp[:, 4 * blk:4 * blk + 4, :], ob[:])
    nc.compile()
    return nc


def _get_nc():
    global _NC
    if _NC is None:
        _NC = _build_nc()
        # The custom-call lowering re-serializes the (immutable, already
        # compiled) BIR module on every dispatch (~60ms); serve it cached.
        blob = _NC.to_json_bytes()
        _NC.to_json_bytes = lambda: blob
    return _NC


def _prep_sample(x1s, x2s, masks):
    """Host prep for one sample: shards + vectors (all cheap)."""
    x1tp = np.pad(x1s * 0.25, ((0, 0), (1, 1), (1, 1))).transpose(1, 2, 0)
    x1fl = np.ascontiguousarray(x1tp).astype(BF16).reshape(-1)
    x2p = np.pad(x2s, ((0, 0), (1, 1), (1, 1))).astype(BF16)

    sq = np.pad((x2s * x2s).sum(0), 1)
    n2 = np.zeros((H2, H2), np.float32)
    mp = np.pad(masks, 1)
    ps = np.zeros((H2, H2), np.float32)
    for u in range(3):
        for v in range(3):
            n2 += sq[u:u + H2, v:v + H2]
            ps += mp[u:u + H2, v:v + H2]
    norm = np.sqrt(n2).reshape(-1)
    mm = (ps.reshape(-1) == 0.0).astype(np.float32)
    scalev = (SCALE * mm / np.maximum(norm, EPS_NORM)).astype(np.float32)
    return x1fl, x2p, scalev, mm


def kernel(x1, x2, mask, fw0, fb0, fw1, fb1, fw2, fb2, fw3, fb3):
    from concourse.bass_utils import run_bass_kernel_spmd

    x1 = np.asarray(x1, np.float32)
    x2 = np.asarray(x2, np.float32)
    mask = np.asarray(mask, np.float32)
    fws = [np.asarray(f, np.float32) for f in (fw0, fw1, fw2, fw3)]
    fbs = [np.asarray(f, np.float32) for f in (fb0, fb1, fb2, fb3)]

    fwt = []
    for q, r in enumerate(RATES):
        t = np.zeros((NSH, 128, 16), np.float32)
        for si, (dh, dv) in enumerate(SHIFTS):
            if dh in (-r, 0, r) and dv in (-r, 0, r):
                u, v = dh // r + 1, dv // r + 1
                t[si] = fws[q][:, :, u, v].T
        fwt.append(t.astype(BF16))

    nc = _get_nc()
    in_maps = []
    for s in range(B):
        x1fl, x2p, scalev, mm = _prep_sample(x1[s], x2[s], mask[s, 0])
        mmbf = mm.astype(BF16)
        for q in range(4):
            sc_hi = scalev.astype(BF16)
            sc_lo = (scalev - sc_hi.astype(np.float32)).astype(BF16)
            fb_hi = fbs[q].astype(BF16)
            fb_lo = (fbs[q] - fb_hi.astype(np.float32)).astype(BF16)
            blob = np.concatenate([
                x1fl[q * X1CH:(q + 1) * X1CH],
                np.ascontiguousarray(x2p[:, 16 * q:16 * q + 18, :]).reshape(-1),
                mmbf,
                fwt[q].reshape(-1),
                sc_hi, sc_lo, fb_hi, fb_lo,
            ])
            in_maps.append({"p_bf": blob})

    res = run_bass_kernel_spmd(nc, in_maps, core_ids=list(range(8)))

    out = np.empty((B, 64, H1, H1), np.float32)
    for s in range(B):
        for q in range(4):
            out[s, 16 * q:16 * (q + 1)] = res.results[4 * s + q]["outp"].astype(np.float32)
    return out


# revision 8
# speedup vs baseline: 1.3257x; 1.3257x over previous
"""Distributed Trainium2 Bass kernel for AtnConv (contextual-attention conv).

Everything runs on device; the tunnel carries only compact inputs and the
final output. 8 cores = batch(2) x quarter(4). Within a sample group of 4:
  - x1^T and x2 (bf16, padded) are uploaded as quarter-shards and AllGathered
    device-side (HBM-HBM over NeuronLink).
  - Each core owns 1024 of the 4096 positions: scores = cols_q^T @ cols,
    scaled in f32 by SCALE*mm/norm, local softmax over all L, exact mask
    multiply + 1e-8 clamp on device.
  - U[c',pos] = R'^T Y via PE (R' streamed straight out of gathered x1^T, so
    col2im consumes U blocks per (di,dj) with no reshuffle), scatter-added
    into a 36-row window; windows AllGathered, every core assembles full y.
  - Final 4 dilated convs: 33-shift union with per-core weight data (zeros
    for foreign rates) keeps the program SPMD-uniform; each core emits only
    its rate's 16 channels [16,128,128] bf16.
Host does only padding/transpose/casts and output concat.
"""

import numpy as np
import ml_dtypes


def _enable_jax_compilation_cache():
    # run_bass_kernel_spmd builds a fresh jit closure per call, so JAX's
    # in-process executable cache never hits and every dispatch re-runs the
    # BIR->NEFF compile (~0.8s). The persistent cache keys on the (stable)
    # serialized HLO and skips that.
    try:
        import jax
        jax.config.update("jax_compilation_cache_dir", "/tmp/jax_comp_cache")
        jax.config.update("jax_persistent_cache_min_compile_time_secs", 0)
        jax.config.update("jax_persistent_cache_min_entry_size_bytes", -1)
    except Exception:
        pass


_enable_jax_compilation_cache()

B, C, H1, H2 = 2, 128, 128, 64
L = H2 * H2            # 4096 patches / positions
POSL = 1024            # positions per core
SCALE = 10.0
EPS_NORM = 1e-4
EPS_CLAMP = 1e-8
RATES = (1, 2, 4, 8)
SHIFTS = sorted({(r * (u - 1), r * (v - 1))
                 for r in RATES for u in range(3) for v in range(3)})
NSH = len(SHIFTS)      # 33
BF16 = ml_dtypes.bfloat16
GROUPS = [[0, 1, 2, 3], [4, 5, 6, 7]]

X1CH = 130 * 130 * 128 // 8   # 270400 bf16-viewed elems per int8 x1 shard
X2QCH = 128 * 18 * 66         # one overlapping 18-row x2 chunk (halo 1)
# bf16 blob layout (element offsets); x1 travels as int8 byte-pairs
OFF_X1 = 0
OFF_X2Q = OFF_X1 + X1CH
OFF_MM = OFF_X2Q + X2QCH
OFF_FW = OFF_MM + L
OFF_SCHI = OFF_FW + NSH * 128 * 16
OFF_SCLO = OFF_SCHI + L
OFF_FBHI = OFF_SCLO + L
OFF_FBLO = OFF_FBHI + 16
BFBLOB = OFF_FBLO + 16

_NC = None


def _build_nc():
    import concourse.bass as bass
    import concourse.bacc as bacc
    import concourse.mybir as mybir
    from concourse import tile

    bf = mybir.dt.bfloat16
    f32 = mybir.dt.float32
    i8 = mybir.dt.int8
    Exp = mybir.ActivationFunctionType.Exp
    Relu = mybir.ActivationFunctionType.Relu
    X = mybir.AxisListType.X
    AG = "AllGather"
    BYP = mybir.AluOpType.bypass

    nc = bacc.Bacc(None, target_bir_lowering=False)
    p_bf = nc.declare_dram_parameter("p_bf", [BFBLOB], bf, isOutput=False)
    outp = nc.declare_dram_parameter("outp", [16, 128, 128], bf, isOutput=True)

    with tile.TileContext(nc) as tc:
        with (
            tc.tile_pool(name="dram", bufs=1, space="DRAM") as dram,
            tc.tile_pool(name="st", bufs=1) as st,
            tc.tile_pool(name="fin", bufs=2) as fin,
        ):
            # ---- kick off input gathers first (overlap with local prep) ----
            b_x2q = dram.tile([128, 18, 66], bf)
            g_x2q = dram.tile([4, 128, 18, 66], bf)
            b_x1 = dram.tile([2 * X1CH], i8)
            g_x1r = dram.tile([8 * X1CH], i8)
            g_x1 = dram.tile([130, 130, 128], bf)
            b_w = dram.tile([128, 36, 130], f32)
            g_w = dram.tile([4, 128, 36, 130], f32)
            d_fw = dram.tile([NSH, 128, 16], bf)
            nc.gpsimd.dma_start(b_x2q[:], p_bf[OFF_X2Q:OFF_X2Q + X2QCH])
            nc.gpsimd.collective_compute(AG, BYP, replica_groups=GROUPS,
                                         ins=[b_x2q[:]], outs=[g_x2q[:]])
            nc.gpsimd.dma_start(b_x1[:],
                                p_bf[OFF_X1:OFF_X1 + X1CH].bitcast(i8))
            nc.gpsimd.dma_start(d_fw[:], p_bf[OFF_FW:OFF_FW + NSH * 128 * 16])
            nc.gpsimd.collective_compute(AG, BYP, replica_groups=GROUPS,
                                         ins=[b_x1[:]], outs=[g_x1r[:]])

            # dequantize gathered int8 x1 -> bf16 (scale is folded into fw
            # host-side; this is a pure convert)
            with tc.tile_pool(name="cvt", bufs=2) as cvt:
                for t in range(5):
                    ci = cvt.tile([128, 3380], i8, tag="ci")
                    nc.sync.dma_start(ci[:], g_x1r[432640 * t:432640 * (t + 1)])
                    cb = cvt.tile([128, 3380], bf, tag="cb")
                    nc.vector.tensor_copy(cb[:], ci[:])
                    nc.sync.dma_start(g_x1[26 * t:26 * t + 26], cb[:])

            # ---- persistent small state ----
            nbmaxs = st.tile([128, 8, 8], f32)
            rss = st.tile([128, 8, 8], f32)
            mmb = st.tile([128, L], bf)
            nc.sync.dma_start(mmb[0:1, :], p_bf[OFF_MM:OFF_MM + L])
            p = 1
            while p < 128:
                nc.sync.dma_start(mmb[p:2 * p, :], mmb[0:p, :])
                p *= 2

            with tc.tile_pool(name="estp", bufs=1) as estp:
                estore = estp.tile([128, 8, L], bf)   # Y^T, 64 KiB/part

                # ---- scores + block-local softmax ----
                with (
                    tc.tile_pool(name="ph1", bufs=1) as ph1,
                    tc.tile_pool(name="wka", bufs=2) as wka,
                    tc.tile_pool(name="psa", bufs=2, space=bass.MemorySpace.PSUM) as psa,
                ):
                    xt = ph1.tile([128, 9, 16, 64], bf)
                    scb = ph1.tile([128, L], f32)
                    for u in range(3):
                        for v in range(3):
                            nc.sync.dma_start(xt[:, 3 * u + v],
                                              b_x2q[:, u:u + 16, v:v + 64])
                    sc_hi = ph1.tile([1, L], bf)
                    sc_lo = ph1.tile([1, L], bf)
                    nc.sync.dma_start(sc_hi[:], p_bf[OFF_SCHI:OFF_SCHI + L])
                    nc.sync.dma_start(sc_lo[:], p_bf[OFF_SCLO:OFF_SCLO + L])
                    nc.vector.tensor_add(scb[0:1, :], sc_hi[:], sc_lo[:])
                    p = 1
                    while p < 128:
                        nc.sync.dma_start(scb[p:2 * p, :], scb[0:p, :])
                        p *= 2

                    for n in range(8):            # L blocks of 512 (8 i-rows)
                        a_n = wka.tile([128, 9, 8, 64], bf, tag="a_n")
                        ch = n // 2
                        r0 = 8 * n - 16 * ch
                        for u in range(3):
                            for v in range(3):
                                nc.sync.dma_start(
                                    a_n[:, 3 * u + v],
                                    g_x2q[ch][:, r0 + u:r0 + u + 8, v:v + 64])
                        for m in range(8):        # pos tiles of 128
                            z = psa.tile([128, 512], f32, tag="z")
                            for k in range(9):
                                nc.tensor.matmul(z[:], xt[:, k, 2 * m:2 * m + 2, :],
                                                 a_n[:, k], start=(k == 0),
                                                 stop=(k == 8))
                            zs = wka.tile([128, 512], f32, tag="zs")
                            nc.vector.tensor_mul(zs[:], z[:],
                                                 scb[:, n * 512:(n + 1) * 512])
                            nc.vector.reduce_max(nbmaxs[:, m, n:n + 1], zs[:],
                                                 axis=X, negate=True)
                            ef = wka.tile([128, 512], f32, tag="ef")
                            nc.scalar.activation(ef[:], zs[:], Exp,
                                                 bias=nbmaxs[:, m, n:n + 1],
                                                 scale=1.0)
                            nc.vector.reduce_sum(rss[:, m, n:n + 1], ef[:], axis=X)
                            nc.vector.tensor_copy(
                                estore[:, m, n * 512:(n + 1) * 512], ef[:])

                # ---- softmax finalize + exact mask & clamp ----
                for m in range(8):
                    ngm = fin.tile([128, 1], f32, tag="ngm")
                    nc.vector.tensor_reduce(ngm[:], nbmaxs[:, m, :], axis=X,
                                            op=mybir.AluOpType.min)
                    al = fin.tile([128, 8], f32, tag="al")
                    nc.scalar.activation(al[:], nbmaxs[:, m, :], Exp, bias=ngm[:],
                                         scale=-1.0)
                    pr = fin.tile([128, 8], f32, tag="pr")
                    nc.vector.tensor_mul(pr[:], al[:], rss[:, m, :])
                    sm = fin.tile([128, 1], f32, tag="sm")
                    nc.vector.reduce_sum(sm[:], pr[:], axis=X)
                    rc = fin.tile([128, 1], f32, tag="rc")
                    nc.vector.reciprocal(rc[:], sm[:])
                    be = fin.tile([128, 8], f32, tag="be")
                    nc.vector.tensor_scalar_mul(be[:], al[:], rc[:])
                    for n in range(8):
                        nc.vector.tensor_scalar_mul(
                            estore[:, m, n * 512:(n + 1) * 512],
                            estore[:, m, n * 512:(n + 1) * 512], be[:, n:n + 1])
                    nc.vector.tensor_mul(estore[:, m, :], estore[:, m, :], mmb[:])
                    nc.vector.tensor_scalar_max(estore[:, m, :], estore[:, m, :],
                                                EPS_CLAMP)

                # ---- U = R'^T Y per pos-half, col2im into window ----
                with tc.tile_pool(name="wpool", bufs=1) as wpool:
                    window = wpool.tile([128, 36, 130], f32)
                    nc.vector.memset(window[:], 0.0)
                    for half in range(2):
                        with (
                            tc.tile_pool(name="ybh", bufs=1) as ybh,
                            tc.tile_pool(name="wkc", bufs=2) as wkc,
                            tc.tile_pool(name="psb", bufs=1,
                                         space=bass.MemorySpace.PSUM) as psb,
                        ):
                            ybufT = ybh.tile([128, 32, 512], bf)
                            for mloc in range(4):
                                m = 4 * half + mloc
                                for kk in range(32):
                                    nc.sync.dma_start_transpose(
                                        ybufT[:, kk, mloc * 128:(mloc + 1) * 128],
                                        estore[:, m, kk * 128:(kk + 1) * 128])
                            for gg in range(4):
                                ups = [psb.tile([128, 8, 64], f32, tag=f"u{j}",
                                                name=f"ups{j}")
                                       for j in range(4)]
                                for k in range(32):
                                    rt = wkc.tile([128, 4, 128], bf, tag="rt")
                                    for j in range(4):
                                        g = 4 * gg + j
                                        di, dj = divmod(g, 4)
                                        nc.sync.dma_start(
                                            rt[:, j],
                                            g_x1[4 * k + di:4 * k + di + 3:2,
                                                 dj:dj + 127:2, :])
                                    for j in range(4):
                                        nc.tensor.matmul(ups[j][:], rt[:, j],
                                                         ybufT[:, k, :],
                                                         start=(k == 0),
                                                         stop=(k == 31))
                                for j in range(4):
                                    g = 4 * gg + j
                                    di, dj = divmod(g, 4)
                                    r0 = di + 1 + 16 * half
                                    sl = window[:, r0:r0 + 15:2, dj:dj + 127:2]
                                    nc.vector.tensor_add(sl, sl, ups[j][:])
                    nc.gpsimd.dma_start(b_w[:], window[:])

            # ---- gather windows, assemble y, final dilated convs ----
            nc.gpsimd.collective_compute(AG, BYP, replica_groups=GROUPS,
                                         ins=[b_w[:]], outs=[g_w[:]])
            with (
                tc.tile_pool(name="convp", bufs=1) as convp,
                tc.tile_pool(name="wkd", bufs=2) as wkd,
                tc.tile_pool(name="psc", bufs=2,
                             space=bass.MemorySpace.PSUM) as psc,
            ):
                y_bf = convp.tile([128, 144, 144], bf)
                fw_sb = convp.tile([128, NSH, 16], bf)
                fb_sb = convp.tile([16, 1], f32)
                for si in range(NSH):
                    nc.sync.dma_start(fw_sb[:, si, :], d_fw[si])
                fb_hi = convp.tile([16, 1], bf)
                fb_lo = convp.tile([16, 1], bf)
                nc.sync.dma_start(fb_hi[:], p_bf[OFF_FBHI:OFF_FBHI + 16])
                nc.sync.dma_start(fb_lo[:], p_bf[OFF_FBLO:OFF_FBLO + 16])
                nc.vector.tensor_add(fb_sb[:], fb_hi[:], fb_lo[:])
                with tc.tile_pool(name="ypool", bufs=1) as ypool:
                    y_buf = ypool.tile([128, 144, 144], f32)
                    nc.vector.memset(y_buf[:], 0.0)
                    for k in range(4):
                        wstg = wkd.tile([128, 36, 130], f32, tag="wstg")
                        nc.gpsimd.dma_start(wstg[:], g_w[k])
                        t0 = 2 if k == 0 else 1
                        t1 = 34 if k == 3 else 35
                        dst = y_buf[:, 32 * k + 6 + t0:32 * k + 6 + t1, 8:136]
                        nc.vector.tensor_add(dst, dst, wstg[:, t0:t1, 1:129])
                    nc.vector.tensor_copy(y_bf[:], y_buf[:])
                for blk in range(32):             # out row blocks of 4
                    ops = psc.tile([16, 4, 128], f32, tag="ops")
                    for si, (dh, dv) in enumerate(SHIFTS):
                        r0 = 8 + dh + 4 * blk
                        nc.tensor.matmul(ops[:], fw_sb[:, si, :],
                                         y_bf[:, r0:r0 + 4, 8 + dv:8 + dv + 128],
                                         start=(si == 0), stop=(si == NSH - 1))
                    ob = wkd.tile([16, 4, 128], bf, tag="ob")
                    nc.scalar.activation(ob[:], ops[:], Relu, bias=fb_sb[:],
                                         scale=1.0)
                    nc.sync.dma_start(outp[:, 4 * blk:4 * blk + 4, :], ob[:])
    nc.compile()
    return nc


def _get_nc():
    global _NC
    if _NC is None:
        _NC = _build_nc()
        # The custom-call lowering re-serializes the (immutable, already
        # compiled) BIR module on every dispatch (~60ms); serve it cached.
        blob = _NC.to_json_bytes()
        _NC.to_json_bytes = lambda: blob
    return _NC


def _prep_sample(x1s, x2s, masks):
    """Host prep for one sample: shards + vectors (all cheap)."""
    x1tp = np.ascontiguousarray(
        np.pad(x1s * 0.25, ((0, 0), (1, 1), (1, 1))).transpose(1, 2, 0),
        np.float32)
    amax = float(np.abs(x1tp).max())
    step = min(amax, 4.0 * float(x1tp.std())) / 127.0
    if step == 0.0:
        step = 1.0
    x1q = np.clip(np.rint(x1tp / step), -127, 127).astype(np.int8)
    x1fl = x1q.reshape(-1).view(BF16)
    x2p = np.pad(x2s, ((0, 0), (1, 1), (1, 1))).astype(BF16)

    sq = np.pad((x2s * x2s).sum(0), 1)
    n2 = np.zeros((H2, H2), np.float32)
    mp = np.pad(masks, 1)
    ps = np.zeros((H2, H2), np.float32)
    for u in range(3):
        for v in range(3):
            n2 += sq[u:u + H2, v:v + H2]
            ps += mp[u:u + H2, v:v + H2]
    norm = np.sqrt(n2).reshape(-1)
    mm = (ps.reshape(-1) == 0.0).astype(np.float32)
    scalev = (SCALE * mm / np.maximum(norm, EPS_NORM)).astype(np.float32)
    return x1fl, x2p, scalev, mm, step


def kernel(x1, x2, mask, fw0, fb0, fw1, fb1, fw2, fb2, fw3, fb3):
    from concourse.bass_utils import run_bass_kernel_spmd

    x1 = np.asarray(x1, np.float32)
    x2 = np.asarray(x2, np.float32)
    mask = np.asarray(mask, np.float32)
    fws = [np.asarray(f, np.float32) for f in (fw0, fw1, fw2, fw3)]
    fbs = [np.asarray(f, np.float32) for f in (fb0, fb1, fb2, fb3)]

    nc = _get_nc()
    in_maps = []
    for s in range(B):
        x1fl, x2p, scalev, mm, step = _prep_sample(x1[s], x2[s], mask[s, 0])
        fwt = []
        for q, r in enumerate(RATES):
            t = np.zeros((NSH, 128, 16), np.float32)
            for si, (dh, dv) in enumerate(SHIFTS):
                if dh in (-r, 0, r) and dv in (-r, 0, r):
                    u, v = dh // r + 1, dv // r + 1
                    t[si] = fws[q][:, :, u, v].T * step
            fwt.append(t.astype(BF16))
        mmbf = mm.astype(BF16)
        for q in range(4):
            sc_hi = scalev.astype(BF16)
            sc_lo = (scalev - sc_hi.astype(np.float32)).astype(BF16)
            fb_hi = fbs[q].astype(BF16)
            fb_lo = (fbs[q] - fb_hi.astype(np.float32)).astype(BF16)
            blob = np.concatenate([
                x1fl[q * X1CH:(q + 1) * X1CH],
                np.ascontiguousarray(x2p[:, 16 * q:16 * q + 18, :]).reshape(-1),
                mmbf,
                fwt[q].reshape(-1),
                sc_hi, sc_lo, fb_hi, fb_lo,
            ])
            in_maps.append({"p_bf": blob})

    res = run_bass_kernel_spmd(nc, in_maps, core_ids=list(range(8)))

    out = np.empty((B, 64, H1, H1), np.float32)
    for s in range(B):
        for q in range(4):
            out[s, 16 * q:16 * (q + 1)] = res.results[4 * s + q]["outp"].astype(np.float32)
    return out
